# revision 9
# baseline (speedup 1.0000x reference)
"""JointAttention TRN2 Bass kernel.

Sharding: 8 cores = batch(2) x head-group(4). Each core owns one batch
element and 4 of the 16 heads (a 256-wide channel slice). Per core:
  qT/kT projections in [c, t] layout (lhsT = W natural, rhs = xT moving),
  v projection in [t, c] layout (lhsT = xT chunk stationary, rhs = W),
  scores^T = K^T.T @ Q^T per 128-key chunk ([k, q] layout, row-tiled
  2 heads at a time), exp on ScalarE, PV with V-augmented-ones columns
  giving the softmax denominators, division via a ones-matmul broadcast,
  and the output projection (row-parallel Wo slice). The 4 partial
  outputs per batch element are summed on the host (row-parallel
  all-reduce as part of unsharding) and bo is added once.
"""

import sys

import numpy as np

if "/opt/trn_rl_repo" not in sys.path:
    sys.path.insert(0, "/opt/trn_rl_repo")

import concourse.bass as bass
import concourse.tile as tile
from concourse import bacc, mybir
from concourse.bass_utils import run_bass_kernel_spmd

F32 = mybir.dt.float32
AFT = mybir.ActivationFunctionType

D = 1024          # model dim
T = 2048          # query length (= self key length)
TK = 4096         # total key length (self + context)
CS = 256          # channels per core (4 heads x 64)
NH = 4            # heads per core
HD = 64           # head dim
DC = 8            # D chunks of 128
N_CORES = 8

# matmul operand dtype: float32 = exact (4 cyc/row), float32r = fast
# (1 cyc/row at N>=256) with reduced-precision multiplies.
MM_DT = F32


def _mm(ap):
    return ap if MM_DT is F32 else ap.bitcast(MM_DT)


def build_nc():
    nc = bacc.Bacc(None)

    xT = nc.declare_dram_parameter("xT", [D, T], F32, isOutput=False)
    cT = nc.declare_dram_parameter("cT", [D, T], F32, isOutput=False)
    wq = nc.declare_dram_parameter("wq", [D, CS], F32, isOutput=False)
    wks = nc.declare_dram_parameter("wks", [D, CS], F32, isOutput=False)
    wkc = nc.declare_dram_parameter("wkc", [D, CS], F32, isOutput=False)
    wvs = nc.declare_dram_parameter("wvs", [D, CS], F32, isOutput=False)
    wvc = nc.declare_dram_parameter("wvc", [D, CS], F32, isOutput=False)
    bq = nc.declare_dram_parameter("bq", [CS, 1], F32, isOutput=False)
    bks = nc.declare_dram_parameter("bks", [CS, 1], F32, isOutput=False)
    bkc = nc.declare_dram_parameter("bkc", [CS, 1], F32, isOutput=False)
    bvs = nc.declare_dram_parameter("bvs", [1, CS], F32, isOutput=False)
    bvc = nc.declare_dram_parameter("bvc", [1, CS], F32, isOutput=False)
    wo = nc.declare_dram_parameter("wo", [CS, D], F32, isOutput=False)
    out = nc.declare_dram_parameter("out", [T, D], F32, isOutput=True)

    with tile.TileContext(nc) as tc:
        _emit(nc, tc, xT, cT, wq, wks, wkc, wvs, wvc,
              bq, bks, bkc, bvs, bvc, wo, out)
    nc.compile()
    return nc


def _emit(nc, tc, xT, cT, wq, wks, wkc, wvs, wvc, bq, bks, bkc, bvs, bvc,
          wo, out):
    from contextlib import ExitStack

    ctx = ExitStack()
    with ctx:
        consts = ctx.enter_context(tc.tile_pool(name="consts", bufs=1))
        w_rot = ctx.enter_context(tc.tile_pool(name="w_rot", bufs=1))
        io_pool = ctx.enter_context(tc.tile_pool(name="io", bufs=16))
        qt_pool = ctx.enter_context(tc.tile_pool(name="qt", bufs=1))
        kt_pool = ctx.enter_context(tc.tile_pool(name="kt", bufs=1))
        v_pool = ctx.enter_context(tc.tile_pool(name="v", bufs=1))
        p_pool = ctx.enter_context(tc.tile_pool(name="p", bufs=4))
        outt_pool = ctx.enter_context(tc.tile_pool(name="outt", bufs=1))
        stage_pool = ctx.enter_context(tc.tile_pool(name="stage", bufs=3))
        misc_pool = ctx.enter_context(tc.tile_pool(name="misc", bufs=2))
        # PSUM: shared(2) + scores(4) + pv(2) = 8 banks
        ps_shared = ctx.enter_context(
            tc.tile_pool(name="ps_shared", bufs=2, space="PSUM"))
        ps_scores = ctx.enter_context(
            tc.tile_pool(name="ps_scores", bufs=4, space="PSUM"))
        ps_pv = ctx.enter_context(
            tc.tile_pool(name="ps_pv", bufs=2, space="PSUM"))

        # ---- constants ----
        w_sb = {}
        t = consts.tile([128, DC, CS], F32, tag="w_wq", name="w_wq")
        nc.sync.dma_start(out=t, in_=wq.rearrange("(a p) c -> p a c", p=128))
        w_sb["wq"] = t
        b_sb = {}
        for name, b in (("bq", bq), ("bks", bks), ("bkc", bkc)):
            t = consts.tile([128, 2], F32, tag=f"b_{name}", name=f"b_{name}")
            nc.sync.dma_start(out=t, in_=b.rearrange("(a p) o -> p (a o)", p=128))
            b_sb[name] = t
        bv_sb = {}
        for name, b in (("bvs", bvs), ("bvc", bvc)):
            t = consts.tile([128, CS], F32, tag=f"bv_{name}", name=f"bv_{name}")
            nc.sync.dma_start(out=t, in_=b[:, :].to_broadcast([128, CS]))
            bv_sb[name] = t
        wo_sb = consts.tile([128, 2, D], F32, tag="wo")
        nc.sync.dma_start(out=wo_sb, in_=wo.rearrange("(a p) f -> p a f", p=128))
        ones_sb = consts.tile([128, HD], F32, tag="ones")
        nc.vector.memset(ones_sb, 1.0)

        # ---- projections ----
        qT_sb = [qt_pool.tile([128, T], F32, tag=f"qT{cc}", name=f"qT{cc}")
                 for cc in range(2)]
        kT_sb = [kt_pool.tile([128, TK], F32, tag=f"kT{cc}", name=f"kT{cc}")
                 for cc in range(2)]
        v_sb = [v_pool.tile([128, NH * (HD + 1)], F32, tag=f"v{kc}", name=f"v{kc}")
                for kc in range(32)]

        for src_i, (src, wk_n, wv_n, bk_n, bv_n) in enumerate((
                (xT, "wks", "wvs", "bks", "bvs"),
                (cT, "wkc", "wvc", "bkc", "bvc"))):
            # rotating weight tiles (self weights, then ctx weights)
            for name, w, wtag in ((wk_n, (wks, wkc)[src_i], "wk"),
                                  (wv_n, (wvs, wvc)[src_i], "wv")):
                t = w_rot.tile([128, DC, CS], F32, tag=wtag, name=f"w_{name}")
                nc.sync.dma_start(out=t, in_=w.rearrange("(a p) c -> p a c", p=128))
                w_sb[name] = t
            # projections, emitted wave-major (one 512-wide t-column of the
            # source at a time) so io pool slots rotate without cycles
            projs = [(wk_n, bk_n, kT_sb, src_i * T)]
            if src_i == 0:
                projs.append(("wq", "bq", qT_sb, 0))
            for tc4 in range(4):
                wave = []
                for dc in range(DC):
                    t = io_pool.tile([128, 512], F32, tag="io",
                                     name=f"io_{src_i}_{tc4}_{dc}")
                    nc.sync.dma_start(
                        out=t,
                        in_=src[dc * 128:(dc + 1) * 128,
                                tc4 * 512:(tc4 + 1) * 512])
                    wave.append(t)

                # q^T / k^T projections: [c, t] layout
                for wn, bn, dst, coff in projs:
                    for cc in range(2):
                        ps = ps_shared.tile([128, 512], F32, tag="ps")
                        for dc in range(DC):
                            nc.tensor.matmul(
                                ps,
                                _mm(w_sb[wn][:, dc, cc * 128:(cc + 1) * 128]),
                                _mm(wave[dc]),
                                start=(dc == 0), stop=(dc == DC - 1))
                        nc.scalar.activation(
                            dst[cc][:, coff + tc4 * 512:coff + (tc4 + 1) * 512],
                            ps, AFT.Identity, bias=b_sb[bn][:, cc:cc + 1])

                # v projection: [t, c] layout, per 128-key chunk
                for sub in range(4):
                    tc_i = tc4 * 4 + sub
                    ps = ps_shared.tile([128, 512], F32, tag="ps")
                    for dc in range(DC):
                        nc.tensor.matmul(
                            ps[:, 0:CS],
                            _mm(wave[dc][:, sub * 128:(sub + 1) * 128]),
                            _mm(w_sb[wv_n][:, dc, :]),
                            start=(dc == 0), stop=(dc == DC - 1))
                    vt = v_sb[src_i * 16 + tc_i]
                    for h in range(NH):
                        nc.vector.tensor_add(
                            vt[:, h * 65:h * 65 + 64],
                            ps[:, h * 64:(h + 1) * 64],
                            bv_sb[bv_n][:, h * 64:(h + 1) * 64])
                    nc.vector.memset(
                        vt[:].rearrange("p (h x) -> p h x", h=NH)[:, :, 64:65],
                        1.0)

        # ---- attention ----
        outT_sb = [outt_pool.tile([128, T], F32, tag=f"outT{cc}", name=f"outT{cc}")
                   for cc in range(2)]
        for qc in range(4):
            qs = slice(qc * 512, (qc + 1) * 512)
            for pair in range(2):
                hA, hB = 2 * pair, 2 * pair + 1
                pvA = ps_pv.tile([128, 512], F32, tag="pv")
                pvB = ps_pv.tile([128, 512], F32, tag="pv")
                for kc in range(32):
                    ks = slice(kc * 128, (kc + 1) * 128)
                    sA = ps_scores.tile([128, 512], F32, tag="s")
                    sB = ps_scores.tile([128, 512], F32, tag="s")
                    nc.tensor.matmul(
                        sA, _mm(kT_sb[pair][0:64, ks]),
                        _mm(qT_sb[pair][0:64, qs]), start=True, stop=True)
                    nc.tensor.matmul(
                        sB, _mm(kT_sb[pair][64:128, ks]),
                        _mm(qT_sb[pair][64:128, qs]), start=True, stop=True,
                        tile_position=(64, 0))
                    pA = p_pool.tile([128, 512], F32, tag="pt")
                    pB = p_pool.tile([128, 512], F32, tag="pt")
                    nc.scalar.activation(pA, sA, AFT.Exp)
                    nc.scalar.activation(pB, sB, AFT.Exp)
                    st, sp = kc == 0, kc == 31
                    vt = v_sb[kc]
                    # head A (even): rows 0-63, denominator row 64
                    nc.tensor.matmul(
                        pvA[0:64, :], _mm(vt[:, hA * 65:hA * 65 + 64]),
                        _mm(pA), start=st, stop=sp)
                    nc.tensor.matmul(
                        pvA[64:65, :], _mm(vt[:, hA * 65 + 64:hA * 65 + 65]),
                        _mm(pA), start=st, stop=sp, tile_position=(0, 64))
                    # head B (odd): rows 64-127, denominator row 0
                    nc.tensor.matmul(
                        pvB[64:128, :], _mm(vt[:, hB * 65:hB * 65 + 64]),
                        _mm(pB), start=st, stop=sp, tile_position=(0, 64))
                    nc.tensor.matmul(
                        pvB[0:1, :], _mm(vt[:, hB * 65 + 64:hB * 65 + 65]),
                        _mm(pB), start=st, stop=sp)
                # softmax division epilogue
                rt = misc_pool.tile([128, 512], F32, tag="recip")
                nc.vector.reciprocal(rt[64:65, :], pvA[64:65, :])
                nc.vector.reciprocal(rt[0:1, :], pvB[0:1, :])
                pvs = misc_pool.tile([128, 512], F32, tag="pvs")
                nc.vector.tensor_copy(pvs[0:64, :], pvA[0:64, :])
                nc.vector.tensor_copy(pvs[64:128, :], pvB[64:128, :])
                bcA = ps_shared.tile([128, 512], F32, tag="ps")
                nc.tensor.matmul(
                    bcA[0:64, :], _mm(ones_sb[64:65, 0:64]), _mm(rt[64:65, :]),
                    start=True, stop=True)
                nc.vector.tensor_mul(
                    outT_sb[pair][0:64, qs], pvs[0:64, :], bcA[0:64, :])
                bcB = ps_shared.tile([128, 512], F32, tag="ps")
                nc.tensor.matmul(
                    bcB[64:128, :], _mm(ones_sb[0:1, 0:64]), _mm(rt[0:1, :]),
                    start=True, stop=True, tile_position=(0, 64))
                nc.vector.tensor_mul(
                    outT_sb[pair][64:128, qs], pvs[64:128, :], bcB[64:128, :])

        # ---- output projection (partial; host sums across head groups) ----
        for qt in range(16):
            qsl = slice(qt * 128, (qt + 1) * 128)
            for fc in range(2):
                fsl = slice(fc * 512, (fc + 1) * 512)
                ps = ps_scores.tile([128, 512], F32, tag="s")
                for cc in range(2):
                    nc.tensor.matmul(
                        ps, _mm(outT_sb[cc][:, qsl]), _mm(wo_sb[:, cc, fsl]),
                        start=(cc == 0), stop=(cc == 1))
                st = stage_pool.tile([128, 512], F32, tag="stage")
                nc.scalar.activation(st, ps, AFT.Copy)
                nc.sync.dma_start(out=out[qsl, fsl], in_=st)


_NC_CACHE = None


def kernel(**inputs):
    global _NC_CACHE
    if _NC_CACHE is None:
        _NC_CACHE = build_nc()
    nc = _NC_CACHE

    f = {k: np.asarray(v, dtype=np.float32) for k, v in inputs.items()}
    x, context = f["x"], f["context"]
    B = x.shape[0]

    xTs = [np.ascontiguousarray(x[b].T) for b in range(B)]
    cTs = [np.ascontiguousarray(context[b].T) for b in range(B)]

    in_maps = []
    for b in range(B):
        for hg in range(4):
            sl = slice(hg * CS, (hg + 1) * CS)
            in_maps.append({
                "xT": xTs[b],
                "cT": cTs[b],
                "wq": np.ascontiguousarray(f["Wq"][:, sl]) * 0.125,
                "wks": np.ascontiguousarray(f["Wks"][:, sl]),
                "wkc": np.ascontiguousarray(f["Wkc"][:, sl]),
                "wvs": np.ascontiguousarray(f["Wvs"][:, sl]),
                "wvc": np.ascontiguousarray(f["Wvc"][:, sl]),
                "bq": (f["bq"][sl] * 0.125).reshape(CS, 1).copy(),
                "bks": f["bks"][sl].reshape(CS, 1).copy(),
                "bkc": f["bkc"][sl].reshape(CS, 1).copy(),
                "bvs": f["bvs"][sl].reshape(1, CS).copy(),
                "bvc": f["bvc"][sl].reshape(1, CS).copy(),
                "wo": np.ascontiguousarray(f["Wo"][sl, :]),
            })

    res = run_bass_kernel_spmd(nc, in_maps, list(range(N_CORES))).results

    bo = f["bo"]
    out = np.empty((B, T, D), dtype=np.float32)
    for b in range(B):
        acc = res[b * 4 + 0]["out"].astype(np.float32).copy()
        for hg in range(1, 4):
            acc += res[b * 4 + hg]["out"]
        out[b] = acc + bo
    return out


# revision 16
# speedup vs baseline: 322.7570x; 322.7570x over previous
"""JointAttention TRN2 Bass kernel.

Sharding: 8 cores = batch(2) x head-group(4). Each core owns one batch
element and 4 of the 16 heads (a 256-wide channel slice). Per core:
  qT/kT projections in [c, t] layout (lhsT = W natural, rhs = xT moving),
  v projection in [t, c] layout (lhsT = xT chunk stationary, rhs = W),
  scores^T = K^T.T @ Q^T per 128-key chunk ([k, q] layout, row-tiled
  2 heads at a time), exp on ScalarE, PV with V-augmented-ones columns
  giving the softmax denominators, division via a ones-matmul broadcast,
  and the output projection (row-parallel Wo slice). The 4 partial
  outputs per batch element are summed on the host (row-parallel
  all-reduce as part of unsharding) and bo is added once.
"""

import sys

import numpy as np

if "/opt/trn_rl_repo" not in sys.path:
    sys.path.insert(0, "/opt/trn_rl_repo")

import concourse.bass as bass
import concourse.tile as tile
from concourse import bacc, mybir
from concourse.bass_utils import run_bass_kernel_spmd

F32 = mybir.dt.float32
AFT = mybir.ActivationFunctionType

D = 1024          # model dim
T = 2048          # query length (= self key length)
TK = 4096         # total key length (self + context)
CS = 256          # channels per core (4 heads x 64)
NH = 4            # heads per core
HD = 64           # head dim
DC = 8            # D chunks of 128
N_CORES = 8

# Storage dtype for every tile that feeds the PE array: float32r runs the
# matmul at 1 cyc/row (vs 4 for float32) with reduced-precision multiplies.
# The BIR verifier requires producers of fp32r matmul operands to emit
# fp32r themselves, so these tiles are allocated as float32r and DRAM-side
# DMA access patterns are bitcast views.
MDT = mybir.dt.float32r


def _dr(ap):
    return ap.bitcast(MDT) if MDT is not F32 else ap


def build_nc():
    nc = bacc.Bacc(None)

    xT = nc.declare_dram_parameter("xT", [D, T], F32, isOutput=False)
    cT = nc.declare_dram_parameter("cT", [D, T], F32, isOutput=False)
    wq = nc.declare_dram_parameter("wq", [D, CS], F32, isOutput=False)
    wks = nc.declare_dram_parameter("wks", [D, CS], F32, isOutput=False)
    wkc = nc.declare_dram_parameter("wkc", [D, CS], F32, isOutput=False)
    wvs = nc.declare_dram_parameter("wvs", [D, CS], F32, isOutput=False)
    wvc = nc.declare_dram_parameter("wvc", [D, CS], F32, isOutput=False)
    bq = nc.declare_dram_parameter("bq", [CS, 1], F32, isOutput=False)
    bks = nc.declare_dram_parameter("bks", [CS, 1], F32, isOutput=False)
    bkc = nc.declare_dram_parameter("bkc", [CS, 1], F32, isOutput=False)
    bvs = nc.declare_dram_parameter("bvs", [1, CS], F32, isOutput=False)
    bvc = nc.declare_dram_parameter("bvc", [1, CS], F32, isOutput=False)
    wo = nc.declare_dram_parameter("wo", [CS, D], F32, isOutput=False)
    ones64 = nc.declare_dram_parameter("ones64", [1, HD], F32, isOutput=False)
    out = nc.declare_dram_parameter("out", [T, D], F32, isOutput=True)

    with tile.TileContext(nc) as tc:
        _emit(nc, tc, xT, cT, wq, wks, wkc, wvs, wvc,
              bq, bks, bkc, bvs, bvc, wo, ones64, out)
    nc.compile()
    return nc


def _emit(nc, tc, xT, cT, wq, wks, wkc, wvs, wvc, bq, bks, bkc, bvs, bvc,
          wo, ones64, out):
    from contextlib import ExitStack

    ctx = ExitStack()
    with ctx:
        consts = ctx.enter_context(tc.tile_pool(name="consts", bufs=1))
        w_rot = ctx.enter_context(tc.tile_pool(name="w_rot", bufs=1))
        io_pool = ctx.enter_context(tc.tile_pool(name="io", bufs=16))
        qt_pool = ctx.enter_context(tc.tile_pool(name="qt", bufs=1))
        kt_pool = ctx.enter_context(tc.tile_pool(name="kt", bufs=1))
        v_pool = ctx.enter_context(tc.tile_pool(name="v", bufs=1))
        p_pool = ctx.enter_context(tc.tile_pool(name="p", bufs=4))
        outt_pool = ctx.enter_context(tc.tile_pool(name="outt", bufs=1))
        stage_pool = ctx.enter_context(tc.tile_pool(name="stage", bufs=3))
        misc_pool = ctx.enter_context(tc.tile_pool(name="misc", bufs=2))
        # PSUM: shared(2) + scores(4) + pv(2) = 8 banks
        ps_shared = ctx.enter_context(
            tc.tile_pool(name="ps_shared", bufs=2, space="PSUM"))
        ps_scores = ctx.enter_context(
            tc.tile_pool(name="ps_scores", bufs=2, space="PSUM"))
        ps_pv = ctx.enter_context(
            tc.tile_pool(name="ps_pv", bufs=2, space="PSUM"))

        # ---- constants ----
        w_sb = {}
        t = consts.tile([128, DC, CS], MDT, tag="w_wq", name="w_wq")
        nc.sync.dma_start(out=t, in_=_dr(wq.rearrange("(a p) c -> p a c", p=128)))
        w_sb["wq"] = t
        b_sb = {}
        for name, b in (("bq", bq), ("bks", bks), ("bkc", bkc)):
            t = consts.tile([128, 2], F32, tag=f"b_{name}", name=f"b_{name}")
            nc.sync.dma_start(out=t, in_=b.rearrange("(a p) o -> p (a o)", p=128))
            b_sb[name] = t
        bv_sb = {}
        for name, b in (("bvs", bvs), ("bvc", bvc)):
            t = consts.tile([128, CS], F32, tag=f"bv_{name}", name=f"bv_{name}")
            nc.sync.dma_start(out=t, in_=b[:, :].to_broadcast([128, CS]))
            bv_sb[name] = t
        wo_sb = consts.tile([128, 2, D], MDT, tag="wo")
        nc.sync.dma_start(out=wo_sb, in_=_dr(wo.rearrange("(a p) f -> p a f", p=128)))
        ones_sb = consts.tile([128, HD], MDT, tag="ones")
        nc.sync.dma_start(out=ones_sb,
                          in_=_dr(ones64[:, :].to_broadcast([128, HD])))

        # ---- projections ----
        qT_sb = [qt_pool.tile([128, T], MDT, tag=f"qT{cc}", name=f"qT{cc}")
                 for cc in range(2)]
        kT_sb = [kt_pool.tile([128, TK], MDT, tag=f"kT{cc}", name=f"kT{cc}")
                 for cc in range(2)]
        v_sb = [v_pool.tile([128, NH * (HD + 1)], MDT, tag=f"v{kc}", name=f"v{kc}")
                for kc in range(32)]

        for src_i, (src, wk_n, wv_n, bk_n, bv_n) in enumerate((
                (xT, "wks", "wvs", "bks", "bvs"),
                (cT, "wkc", "wvc", "bkc", "bvc"))):
            # rotating weight tiles (self weights, then ctx weights)
            for name, w, wtag in ((wk_n, (wks, wkc)[src_i], "wk"),
                                  (wv_n, (wvs, wvc)[src_i], "wv")):
                t = w_rot.tile([128, DC, CS], MDT, tag=wtag, name=f"w_{name}")
                nc.sync.dma_start(out=t, in_=_dr(w.rearrange("(a p) c -> p a c", p=128)))
                w_sb[name] = t
            # projections, emitted wave-major (one 512-wide t-column of the
            # source at a time) so io pool slots rotate without cycles
            projs = [(wk_n, bk_n, kT_sb, src_i * T)]
            if src_i == 0:
                projs.append(("wq", "bq", qT_sb, 0))
            for tc4 in range(4):
                wave = []
                for dc in range(DC):
                    t = io_pool.tile([128, 512], MDT, tag="io",
                                     name=f"io_{src_i}_{tc4}_{dc}")
                    nc.sync.dma_start(
                        out=t,
                        in_=_dr(src[dc * 128:(dc + 1) * 128,
                                    tc4 * 512:(tc4 + 1) * 512]))
                    wave.append(t)

                # q^T / k^T projections: [c, t] layout
                for wn, bn, dst, coff in projs:
                    for cc in range(2):
                        ps = ps_shared.tile([128, 512], F32, tag="ps")
                        for dc in range(DC):
                            nc.tensor.matmul(
                                ps,
                                (w_sb[wn][:, dc, cc * 128:(cc + 1) * 128]),
                                (wave[dc]),
                                start=(dc == 0), stop=(dc == DC - 1))
                        nc.vector.tensor_scalar_add(
                            dst[cc][:, coff + tc4 * 512:coff + (tc4 + 1) * 512],
                            ps, b_sb[bn][:, cc:cc + 1])

                # v projection: [t, c] layout, per 128-key chunk
                for sub in range(4):
                    tc_i = tc4 * 4 + sub
                    ps = ps_shared.tile([128, 512], F32, tag="ps")
                    for dc in range(DC):
                        nc.tensor.matmul(
                            ps[:, 0:CS],
                            (wave[dc][:, sub * 128:(sub + 1) * 128]),
                            (w_sb[wv_n][:, dc, :]),
                            start=(dc == 0), stop=(dc == DC - 1))
                    vt = v_sb[src_i * 16 + tc_i]
                    for h in range(NH):
                        nc.vector.tensor_add(
                            vt[:, h * 65:h * 65 + 64],
                            ps[:, h * 64:(h + 1) * 64],
                            bv_sb[bv_n][:, h * 64:(h + 1) * 64])
                    vt_ones = vt[:].rearrange(
                        "p (h x) -> p h x", h=NH)[:, :, 64:65].rearrange(
                        "p h one -> p (h one)")
                    nc.sync.dma_start(
                        out=vt_ones,
                        in_=_dr(ones64[:, 0:NH].to_broadcast([128, NH])))

        # ---- attention ----
        outT_sb = [outt_pool.tile([128, T], MDT, tag=f"outT{cc}", name=f"outT{cc}")
                   for cc in range(2)]
        for qc in range(4):
            qs = slice(qc * 512, (qc + 1) * 512)
            for pair in range(2):
                hA, hB = 2 * pair, 2 * pair + 1
                pvA = ps_pv.tile([128, 512], F32, tag="pv")
                pvB = ps_pv.tile([128, 512], F32, tag="pv")
                for kc in range(32):
                    ks = slice(kc * 128, (kc + 1) * 128)
                    s2 = ps_scores.tile([128, 1024], F32, tag="s")
                    nc.tensor.matmul(
                        s2[:, 0:512], (kT_sb[pair][0:64, ks]),
                        (qT_sb[pair][0:64, qs]), start=True, stop=True)
                    nc.tensor.matmul(
                        s2[:, 512:1024], (kT_sb[pair][64:128, ks]),
                        (qT_sb[pair][64:128, qs]), start=True, stop=True,
                        tile_position=(64, 0))
                    pt = p_pool.tile([128, 1024], MDT, tag="pt")
                    nc.scalar.activation(pt, s2, AFT.Exp)
                    st, sp = kc == 0, kc == 31
                    vt = v_sb[kc]
                    # [V | ones] stationary: rows 0-63 = head out^T,
                    # row 64 = softmax denominator
                    nc.tensor.matmul(
                        pvA[0:65, :], vt[:, hA * 65:(hA + 1) * 65],
                        pt[:, 0:512], start=st, stop=sp)
                    nc.tensor.matmul(
                        pvB[0:65, :], vt[:, hB * 65:(hB + 1) * 65],
                        pt[:, 512:1024], start=st, stop=sp)
                # softmax division epilogue; the reciprocal rows sit at
                # partition 64 (where the PV denominators land)
                rt = misc_pool.tile([128, 1024], MDT, tag="recip")
                with nc.allow_low_precision(
                        reason="fp32r rounding of softmax reciprocal"):
                    nc.vector.reciprocal(rt[64:65, 0:512], pvA[64:65, :])
                    nc.vector.reciprocal(rt[64:65, 512:1024], pvB[64:65, :])
                pvs = misc_pool.tile([128, 1024], F32, tag="pvs")
                nc.vector.tensor_copy(pvs[0:64, 0:512], pvA[0:64, :])
                nc.vector.tensor_copy(pvs[0:64, 512:1024], pvB[0:64, :])
                bcA = ps_shared.tile([128, 512], F32, tag="ps")
                nc.tensor.matmul(
                    bcA[0:64, :], ones_sb[64:65, 0:64], rt[64:65, 0:512],
                    start=True, stop=True, tile_position=(64, 0))
                nc.vector.tensor_mul(
                    outT_sb[pair][0:64, qs], pvs[0:64, 0:512], bcA[0:64, :])
                bcB = ps_shared.tile([128, 512], F32, tag="ps")
                nc.tensor.matmul(
                    bcB[0:64, :], ones_sb[64:65, 0:64], rt[64:65, 512:1024],
                    start=True, stop=True, tile_position=(64, 0))
                odd = misc_pool.tile([128, 512], MDT, tag="odd")
                nc.vector.tensor_mul(
                    odd[0:64, :], pvs[0:64, 512:1024], bcB[0:64, :])
                # odd head lives at partitions 64-127 of outT: shift via DMA
                nc.sync.dma_start(out=outT_sb[pair][64:128, qs],
                                  in_=odd[0:64, :])

        # ---- output projection (partial; host sums across head groups) ----
        for qt in range(16):
            qsl = slice(qt * 128, (qt + 1) * 128)
            for fc in range(2):
                fsl = slice(fc * 512, (fc + 1) * 512)
                ps = ps_scores.tile([128, 512], F32, tag="s")
                for cc in range(2):
                    nc.tensor.matmul(
                        ps, (outT_sb[cc][:, qsl]), (wo_sb[:, cc, fsl]),
                        start=(cc == 0), stop=(cc == 1))
                st = stage_pool.tile([128, 512], F32, tag="stage")
                nc.vector.tensor_copy(st, ps)
                nc.sync.dma_start(out=out[qsl, fsl], in_=st)


_NC_CACHE = None


def kernel(**inputs):
    global _NC_CACHE
    if _NC_CACHE is None:
        _NC_CACHE = build_nc()
    nc = _NC_CACHE

    f = {k: np.asarray(v, dtype=np.float32) for k, v in inputs.items()}
    x, context = f["x"], f["context"]
    B = x.shape[0]

    xTs = [np.ascontiguousarray(x[b].T) for b in range(B)]
    cTs = [np.ascontiguousarray(context[b].T) for b in range(B)]

    in_maps = []
    for b in range(B):
        for hg in range(4):
            sl = slice(hg * CS, (hg + 1) * CS)
            in_maps.append({
                "xT": xTs[b],
                "cT": cTs[b],
                "wq": np.ascontiguousarray(f["Wq"][:, sl]) * 0.125,
                "wks": np.ascontiguousarray(f["Wks"][:, sl]),
                "wkc": np.ascontiguousarray(f["Wkc"][:, sl]),
                "wvs": np.ascontiguousarray(f["Wvs"][:, sl]),
                "wvc": np.ascontiguousarray(f["Wvc"][:, sl]),
                "bq": (f["bq"][sl] * 0.125).reshape(CS, 1).copy(),
                "bks": f["bks"][sl].reshape(CS, 1).copy(),
                "bkc": f["bkc"][sl].reshape(CS, 1).copy(),
                "bvs": f["bvs"][sl].reshape(1, CS).copy(),
                "bvc": f["bvc"][sl].reshape(1, CS).copy(),
                "wo": np.ascontiguousarray(f["Wo"][sl, :]),
                "ones64": np.ones((1, HD), dtype=np.float32),
            })

    res = run_bass_kernel_spmd(nc, in_maps, list(range(N_CORES))).results

    bo = f["bo"]
    out = np.empty((B, T, D), dtype=np.float32)
    for b in range(B):
        acc = res[b * 4 + 0]["out"].astype(np.float32).copy()
        for hg in range(1, 4):
            acc += res[b * 4 + hg]["out"]
        out[b] = acc + bo
    return out
